# revision 70
# baseline (speedup 1.0000x reference)
"""CRPE sparse attention kernel for 8 Trainium2 NeuronCores.

Strategy (graph/edge parallelism over query-sorted edges):
  * Host sorts the M edges by query index; core c owns all edges whose query
    lies in [1024c, 1024c+1024). Queries are disjoint per core, so the
    segment softmax and output rows are core-local; the host concatenates
    the 8 output slices.
  * P1: each core projects q|k|v (+bias) for its own 1024 rows; the k|v rows
    (augmented with bf16 hi/lo-split x_C coords for exact bin computation)
    are AllGathered so every core holds the full [8192, 1152] k|v table.
  * P2: edges in 128-wide blocks (edge -> SBUF partition), two blocks per
    iteration to halve DVE instruction count:
      - k|v and q rows fetched with dma_gather (2.25KB / 1.25KB bf16 rows),
      - relative-position bins in f32 (exact +2^23 floor trick); one fused
        is_equal builds all per-axis one-hots of a chunk (bins 4..43 only:
        x_C uniform in [0,1) keeps rel/0.05+24 inside [4,44)),
      - PE: transpose one-hots, matmul against the (120x512) tables for
        per-edge T_q/T_k/T_v rows; k rows are folded into T_q in PSUM via an
        identity-matmul accumulate,
      - DVE: logits = q.(k+T_q) + k.T_k per head with an interleaved
        product layout and a single fused reduce,
      - ACT: e = exp(logits + pad_bias); max-subtraction is skipped since
        logits are O(1) by construction and softmax is shift-invariant,
      - block-local segment reduction via one-hot scatter matmuls
        S^T @ [e*(v+T_v) | e] into W query slots; partials go to DRAM.
  * P3: per query, gather its <=KP partial rows, sum, divide by the summed
    denominator (+1e-20 so empty segments yield 0), write the output.
"""

import os

os.environ.setdefault("MYCRO_LOCAL_CACHE", "1")

import numpy as np
import ml_dtypes

import concourse.mybir as mybir
import concourse.bass as bass
import concourse.bacc as bacc
import concourse.tile as tile
from concourse.bass_utils import run_bass_kernel_spmd

F32 = mybir.dt.float32
BF16 = mybir.dt.bfloat16
I16 = mybir.dt.int16
ALU = mybir.AluOpType
ACTF = mybir.ActivationFunctionType
BF16NP = ml_dtypes.bfloat16

N = 8192
M_EDGES = 131072
C_IN = 512
H = 8
D = 64
CH = H * D          # 512
KVW = 2 * CH        # 1024
KVR = 1152          # kv row: k(512) | v(512) | xc_hi(3) | xc_lo(3) | pad -> 2304B
QR = 640            # q row: q(512) | xc_hi(3) | xc_lo(3) | pad -> 1280B
BINS = 48
BIN_SIZE = 0.05
BQ = 40             # effective bins 4..43 (x_C uniform [0,1) keeps rel in (4,44))
NCORES = 8
NQ = N // NCORES    # queries per core
CHUNK = 8           # 128-edge blocks per gather chunk
PAD_BIAS = -30000.0

_CACHE = {}


def _bcast(ap, n):
    """Append a stride-0 dim of size n to an AP (free-dim broadcast)."""
    return bass.AP(ap.tensor, ap.offset, list(ap.ap) + [[0, n]])


def _wrap_idx(a):
    """dma_gather index layout: wrapped in 16 partitions, replicated x8."""
    a = np.asarray(a, np.int16)
    assert len(a) % 16 == 0
    return np.ascontiguousarray(np.tile(a.reshape(-1, 16).T, (8, 1)))


def _build(nb, W, KP):
    """Build + compile the single-core SPMD program for NB=nb edge blocks."""
    mcp = nb * 128
    nchunks = nb // CHUNK
    zr = nb * W  # index of the all-zero partials row

    nc = bacc.Bacc("TRN2", target_bir_lowering=False, debug=False,
                   num_swdge_queues=4, num_devices=NCORES)

    # ---- I/O ----
    xTq = nc.dram_tensor("xTq", [C_IN, NQ], BF16, kind="ExternalInput")
    bqkv = nc.dram_tensor("bqkv", [1, 3 * CH], BF16, kind="ExternalInput")
    wkv = nc.dram_tensor("wkv", [C_IN, KVW], BF16, kind="ExternalInput")
    wq = nc.dram_tensor("wq", [C_IN, CH], BF16, kind="ExternalInput")
    tabq = nc.dram_tensor("tabq", [3 * BQ, CH], BF16, kind="ExternalInput")
    tabk = nc.dram_tensor("tabk", [3 * BQ, CH], BF16, kind="ExternalInput")
    tabv = nc.dram_tensor("tabv", [3 * BQ, CH], BF16, kind="ExternalInput")
    xchlq = nc.dram_tensor("xchlq", [NQ, 8], BF16, kind="ExternalInput")
    iota = nc.dram_tensor("iota", [128, BQ], F32, kind="ExternalInput")
    ident = nc.dram_tensor("ident", [128, 128], BF16, kind="ExternalInput")
    ki_d = nc.dram_tensor("ki", [128, mcp // 16], I16, kind="ExternalInput")
    qil_d = nc.dram_tensor("qil", [128, mcp // 16], I16, kind="ExternalInput")
    piece_d = nc.dram_tensor("piece", [128, NQ * KP // 16], I16, kind="ExternalInput")
    smat = nc.dram_tensor("smat", [nb, 128, W], BF16, kind="ExternalInput")
    ebias_d = nc.dram_tensor("ebias", [128, nb], F32, kind="ExternalInput")
    out_d = nc.dram_tensor("out", [NQ, CH], F32, kind="ExternalOutput")

    # ---- internal DRAM ----
    kv_mine = nc.dram_tensor("kv_m", [NQ, KVR], BF16)
    kv_dram = nc.dram_tensor("kv_i", [N, KVR], BF16, addr_space="Shared")
    q_dram = nc.dram_tensor("q_i", [NQ, QR], BF16)
    part_dram = nc.dram_tensor("part_i", [nb * W + 1, 576], F32)

    with tile.TileContext(nc) as tc:
        with tc.tile_pool(name="const", bufs=1) as cp:
            wkv_sb = cp.tile([128, 4, KVW], BF16)
            nc.sync.dma_start(wkv_sb[:], wkv[:].rearrange("(c p) n -> p c n", p=128))
            wq_sb = cp.tile([128, 4, CH], BF16)
            nc.sync.dma_start(wq_sb[:], wq[:].rearrange("(c p) n -> p c n", p=128))
            tq_sb = cp.tile([3 * BQ, CH], BF16)
            nc.sync.dma_start(tq_sb[:], tabq[:])
            tk_sb = cp.tile([3 * BQ, CH], BF16)
            nc.sync.dma_start(tk_sb[:], tabk[:])
            tv_sb = cp.tile([3 * BQ, CH], BF16)
            nc.sync.dma_start(tv_sb[:], tabv[:])
            iota_sb = cp.tile([128, BQ], F32)
            nc.sync.dma_start(iota_sb[:], iota[:])
            ident_sb = cp.tile([128, 128], BF16)
            nc.sync.dma_start(ident_sb[:], ident[:])
            ki_sb = cp.tile([128, mcp // 16], I16)
            nc.sync.dma_start(ki_sb[:], ki_d[:])
            qil_sb = cp.tile([128, mcp // 16], I16)
            nc.sync.dma_start(qil_sb[:], qil_d[:])
            piece_sb = cp.tile([128, NQ * KP // 16], I16)
            nc.sync.dma_start(piece_sb[:], piece_d[:])
            ebias_sb = cp.tile([128, nb], F32)
            nc.sync.dma_start(ebias_sb[:], ebias_d[:])
            b_sb = cp.tile([1, 3 * CH], BF16)
            nc.sync.dma_start(b_sb[:], bqkv[:])
            ones1 = cp.tile([1, 128], BF16)
            nc.vector.memset(ones1[:], 1.0)

            # ---------------- P1: projections ----------------
            with (
                tc.tile_pool(name="p1sb", bufs=3) as p1,
                tc.tile_pool(name="p1ps", bufs=2, space="PSUM") as p1p,
            ):
                # one fused loop: q|k|v from a single xT load per row tile;
                # q_dram completes before the collective so P2's q gathers
                # can prefetch under it
                for r in range(NQ // 128):
                    xt_t = p1.tile([128, 4, 128], BF16, tag="xt")
                    nc.sync.dma_start(
                        xt_t[:],
                        xTq[:, r * 128:(r + 1) * 128].rearrange(
                            "(c p) m -> p c m", p=128
                        ),
                    )
                    psq = p1p.tile([128, CH], F32, tag="psq")
                    for c in range(4):
                        nc.tensor.matmul(
                            psq[:], xt_t[:, c, :], wq_sb[:, c, :],
                            start=(c == 0), stop=False,
                        )
                    nc.tensor.matmul(psq[:], ones1[:], b_sb[:, 0:CH],
                                     start=False, stop=True)
                    q_sb = p1.tile([128, QR], BF16, tag="qo")
                    # fold the D^-0.5 query scaling into the PSUM->SBUF copy
                    nc.scalar.activation(q_sb[:, 0:CH], psq[:], ACTF.Copy,
                                         scale=float(D) ** -0.5)
                    nc.sync.dma_start(q_sb[:, CH:CH + 8],
                                      xchlq[r * 128:(r + 1) * 128, :])
                    nc.gpsimd.memset(q_sb[:, CH + 8:QR], 0.0)
                    nc.sync.dma_start(q_dram[r * 128:(r + 1) * 128, :], q_sb[:])

                    ps = p1p.tile([128, KVW], F32, tag="pskv")
                    for c in range(4):
                        nc.tensor.matmul(
                            ps[:, 0:CH], xt_t[:, c, :], wkv_sb[:, c, 0:CH],
                            start=(c == 0), stop=False,
                        )
                        nc.tensor.matmul(
                            ps[:, CH:KVW], xt_t[:, c, :], wkv_sb[:, c, CH:KVW],
                            start=(c == 0), stop=False,
                        )
                    # + b_kv broadcast to all rows (K=1 outer product)
                    nc.tensor.matmul(ps[:, 0:CH], ones1[:], b_sb[:, CH:KVW],
                                     start=False, stop=True)
                    nc.tensor.matmul(ps[:, CH:KVW], ones1[:], b_sb[:, KVW:KVW + CH],
                                     start=False, stop=True)
                    kv_sb = p1.tile([128, KVR], BF16, tag="kvo")
                    nc.scalar.copy(kv_sb[:, 0:KVW], ps[:])
                    nc.sync.dma_start(kv_sb[:, KVW:KVW + 8],
                                      xchlq[r * 128:(r + 1) * 128, :])
                    nc.gpsimd.memset(kv_sb[:, KVW + 8:KVR], 0.0)
                    nc.sync.dma_start(kv_mine[r * 128:(r + 1) * 128, :], kv_sb[:])
                # two half AllGathers: the first overlaps the second half of
                # the projection loop. Row layout becomes half-blocked
                # (ki indices are remapped accordingly on the host).
                nc.gpsimd.collective_compute(
                    "AllGather", ALU.bypass,
                    replica_groups=[list(range(NCORES))],
                    ins=[kv_mine[0:NQ // 2, :]], outs=[kv_dram[0:N // 2, :]],
                )
                nc.gpsimd.collective_compute(
                    "AllGather", ALU.bypass,
                    replica_groups=[list(range(NCORES))],
                    ins=[kv_mine[NQ // 2:NQ, :]], outs=[kv_dram[N // 2:N, :]],
                )

            # partials pad columns (520:576) are gathered in P3 but never
            # summed; zero-fill them once so the rows are fully defined
            with tc.tile_pool(name="zf", bufs=1) as zf:
                z56 = zf.tile([128, 56], F32)
                nc.vector.memset(z56[:], 0.0)
                nrows = nb * W + 1
                for r0 in range(0, nrows, 128):
                    n = min(128, nrows - r0)
                    nc.sync.dma_start(part_dram[r0:r0 + n, 520:576], z56[0:n, :])

            # ---------------- P2: edge blocks ----------------
            with (
                tc.tile_pool(name="p2g", bufs=3) as pg_pool,
                tc.tile_pool(name="p2w", bufs=4) as pw,
                tc.tile_pool(name="p2ps", bufs=2, space="PSUM") as pps,
                tc.tile_pool(name="p2ps1", bufs=2, space="PSUM") as pps1,
                tc.tile_pool(name="p2ps2", bufs=1, space="PSUM") as pps2,
            ):
                for g in range(nchunks):
                    i0 = g * CHUNK * 8  # idx column offset (16 idx per column)
                    hc = CHUNK // 2
                    kvg = pg_pool.tile([128, CHUNK, KVR], BF16, tag="kvg")
                    qg = pg_pool.tile([128, CHUNK, QR], BF16, tag="qg")
                    for hf in range(2):
                        nc.gpsimd.dma_gather(
                            kvg[:, hf * hc:(hf + 1) * hc, :], kv_dram[:],
                            ki_sb[:, i0 + hf * hc * 8:i0 + (hf + 1) * hc * 8],
                            hc * 128, hc * 128, KVR, queue_num=0,
                        )
                        nc.gpsimd.dma_gather(
                            qg[:, hf * hc:(hf + 1) * hc, :], q_dram[:],
                            qil_sb[:, i0 + hf * hc * 8:i0 + (hf + 1) * hc * 8],
                            hc * 128, hc * 128, QR, queue_num=0,
                        )
                    s_t = pg_pool.tile([128, CHUNK, W], BF16, tag="smat")
                    nc.sync.dma_start(
                        s_t[:],
                        smat[g * CHUNK:(g + 1) * CHUNK].rearrange("b p w -> p b w"),
                    )

                    # bin indices for the whole chunk, f32-exact floor of
                    # (xcq-xck)/BIN_SIZE + BINS/2 with xc = hi + lo (bf16 pair)
                    d6 = pw.tile([128, CHUNK, 6], F32, tag="d6")
                    nc.vector.tensor_tensor(
                        d6[:], qg[:, :, CH:CH + 6], kvg[:, :, KVW:KVW + 6],
                        op=ALU.subtract,
                    )
                    rel = pw.tile([128, CHUNK, 3], F32, tag="rel")
                    nc.vector.tensor_tensor(
                        rel[:], d6[:, :, 0:3], d6[:, :, 3:6], op=ALU.add)
                    nc.vector.tensor_scalar(
                        rel[:], rel[:], 1.0 / BIN_SIZE, BINS / 2.0,
                        op0=ALU.mult, op1=ALU.add,
                    )
                    flo = pw.tile([128, CHUNK, 3], F32, tag="flo")
                    # exact floor: y = (x + 2^23) - 2^23, then y -= (y > x)
                    nc.vector.tensor_scalar(flo[:], rel[:], float(2 ** 23),
                                            float(2 ** 23), op0=ALU.add,
                                            op1=ALU.subtract)
                    gt = d6[:, :, 0:3]  # d6 is dead after rel; reuse as scratch
                    nc.vector.tensor_tensor(gt, flo[:], rel[:], op=ALU.is_gt)
                    nc.vector.tensor_tensor(flo[:], flo[:], gt, op=ALU.subtract)
                    nc.vector.tensor_scalar(
                        flo[:], flo[:], float(BINS / 2 - BQ / 2), float(BINS / 2 + BQ / 2 - 1),
                        op0=ALU.max, op1=ALU.min,
                    )
                    # all CHUNK x 3 one-hots in a single compare:
                    # ohc[p, b, a*BQ+j] = (flo[p, b, a] == iota[j]);
                    # cols 120:128 are zero padding for the DMA transpose
                    ohc = pw.tile([128, CHUNK, 128], BF16, tag="ohc")
                    flo_b = bass.AP(flo[:].tensor, flo[:].offset,
                                    list(flo[:].ap) + [[0, BQ]])
                    iota_v = bass.AP(iota_sb[:].tensor, iota_sb[:].offset,
                                     [iota_sb[:].ap[0], [0, CHUNK], [0, 3],
                                      [1, BQ]])
                    nc.vector.tensor_tensor(
                        ohc[:, :, 0:3 * BQ].rearrange("p b (a j) -> p b a j", a=3),
                        flo_b, iota_v, op=ALU.is_equal,
                    )
                    nc.gpsimd.memset(ohc[:, :, 3 * BQ:128], 0.0)

                    for b in range(0, CHUNK, 2):
                        blk = g * CHUNK + b
                        oht_ps = pps1.tile([128, 2, 128], BF16, tag="ohtps")
                        for j in range(2):
                            nc.tensor.transpose(
                                oht_ps[0:3 * BQ, j, :], ohc[:, b + j, 0:3 * BQ],
                                ident_sb[:],
                            )
                        oht_f = pw.tile([128, 2, 128], BF16, tag="oht")
                        nc.scalar.copy(oht_f[0:3 * BQ, :, :], oht_ps[0:3 * BQ, :, :])
                        oht = [oht_f[0:3 * BQ, 0, :], oht_f[0:3 * BQ, 1, :]]

                        kv2 = kvg[:, b:b + 2, :]
                        # serially reused 2-bank PSUM pair tile; k rows are
                        # added into T_q on PE via identity-matmul accumulation
                        t_q = pps.tile([128, 2, CH], F32, tag="T")
                        for j in range(2):
                            nc.tensor.matmul(t_q[:, j, :], oht[j], tq_sb[:],
                                             start=True, stop=False)
                            nc.tensor.matmul(t_q[:, j, :], ident_sb[:],
                                             kvg[:, b + j, 0:CH],
                                             start=False, stop=True)
                        t_k = pps.tile([128, 2, CH], F32, tag="T")
                        nc.tensor.matmul(t_k[:, 0, :], oht[0], tk_sb[:])
                        nc.tensor.matmul(t_k[:, 1, :], oht[1], tk_sb[:])
                        tkb = pw.tile([128, 2, CH], BF16, tag="tkb")
                        nc.scalar.copy(tkb[:], t_k[:])
                        tqb = pw.tile([128, 2, CH], BF16, tag="tqb")
                        nc.scalar.copy(tqb[:], t_q[:])
                        # pa/pb interleaved per head -> one fused reduce
                        pab = pw.tile([128, 2, 2 * CH], BF16, tag="pab")
                        pab_v = pab[:].rearrange(
                            "p b (h s d) -> p b h s d", s=2, d=D)
                        nc.vector.tensor_tensor(
                            pab_v[:, :, :, 0, :],
                            qg[:, b:b + 2, 0:CH].rearrange(
                                "p b (h d) -> p b h d", d=D),
                            tqb[:].rearrange("p b (h d) -> p b h d", d=D),
                            op=ALU.mult,
                        )
                        nc.vector.tensor_tensor(
                            pab_v[:, :, :, 1, :],
                            kv2[:, :, 0:CH].rearrange("p b (h d) -> p b h d", d=D),
                            tkb[:].rearrange("p b (h d) -> p b h d", d=D),
                            op=ALU.mult,
                        )
                        lg = pw.tile([128, 2, H], F32, tag="lg")
                        nc.vector.tensor_reduce(
                            lg[:], pab[:].rearrange("p b (h x) -> p b h x", x=2 * D),
                            axis=mybir.AxisListType.X, op=ALU.add,
                        )

                        srhs = pw.tile([128, 2, CH + 8], BF16, tag="srhs")
                        for j in range(2):
                            nc.scalar.activation(
                                srhs[:, j, CH:CH + H], lg[:, j, :], ACTF.Exp,
                                bias=ebias_sb[:, blk + j:blk + j + 1],
                            )
                        t_v = pps.tile([128, 2, CH], F32, tag="T")
                        for j in range(2):
                            nc.tensor.matmul(t_v[:, j, :], oht[j], tv_sb[:],
                                             start=True, stop=False)
                            nc.tensor.matmul(t_v[:, j, :], ident_sb[:],
                                             kvg[:, b + j, CH:KVW],
                                             start=False, stop=True)
                        tvb = pw.tile([128, 2, CH], BF16, tag="tvb")
                        nc.scalar.copy(tvb[:], t_v[:])
                        nc.vector.tensor_tensor(
                            srhs[:, :, 0:CH].rearrange(
                                "p b (h d) -> p b h d", d=D),
                            tvb[:].rearrange("p b (h d) -> p b h d", d=D),
                            _bcast(srhs[:, :, CH:CH + H], D),
                            op=ALU.mult,
                        )

                        for j in range(2):
                            sc = pps2.tile([W, 520], F32, tag="sc")
                            nc.tensor.matmul(sc[:, 0:CH], s_t[:, b + j, :],
                                             srhs[:, j, 0:CH])
                            nc.tensor.matmul(sc[:, CH:CH + H], s_t[:, b + j, :],
                                             srhs[:, j, CH:CH + H])
                            part = pw.tile([W, 520], F32, tag="part")
                            nc.scalar.copy(part[:], sc[:, 0:520])
                            nc.sync.dma_start(
                                part_dram[(blk + j) * W:(blk + j + 1) * W, 0:520],
                                part[:],
                            )

            # ---------------- P3: merge partials, divide ----------------
            with tc.tile_pool(name="p3", bufs=2) as p3:
                z = p3.tile([1, 576], F32, tag="z")
                nc.vector.memset(z[:], 0.0)
                nc.sync.dma_start(part_dram[zr:zr + 1, :], z[:])
                for t in range(NQ // 128):
                    pcs = p3.tile([128, KP, 576], F32, tag="pcs")
                    nc.gpsimd.dma_gather(
                        pcs[:], part_dram[:],
                        piece_sb[:, t * KP * 8:(t + 1) * KP * 8],
                        KP * 128, KP * 128, 576, queue_num=0,
                    )
                    s1 = p3.tile([128, 520], F32, tag="s1")
                    nc.vector.tensor_tensor(
                        s1[:], pcs[:, 0, 0:520], pcs[:, 1, 0:520], op=ALU.add
                    )
                    for j in range(2, KP):
                        nc.vector.tensor_tensor(
                            s1[:], s1[:], pcs[:, j, 0:520], op=ALU.add
                        )
                    rc = p3.tile([128, H], F32, tag="rc")
                    nc.vector.tensor_scalar_add(rc[:], s1[:, CH:CH + H], 1e-20)
                    nc.vector.reciprocal(rc[:], rc[:])
                    o_t = p3.tile([128, CH], F32, tag="o")
                    nc.vector.tensor_tensor(
                        o_t[:].rearrange("p (h d) -> p h d", d=D),
                        s1[:, 0:CH].rearrange("p (h d) -> p h d", d=D),
                        _bcast(rc[:], D),
                        op=ALU.mult,
                    )
                    nc.sync.dma_start(out_d[t * 128:(t + 1) * 128, :], o_t[:])

    nc.compile()
    return nc


def _core_edges(inputs, nb):
    """Sorted, padded per-core (qi_local, ki_global) index arrays."""
    mcp = nb * 128
    pairs = np.asarray(inputs["qk_pair_idxs"])
    qi = pairs[0].astype(np.int64)
    ki = pairs[1].astype(np.int64)
    order = np.argsort(qi, kind="stable")
    qi_s = qi[order]
    ki_s = ki[order]
    starts = np.searchsorted(qi_s, np.arange(0, N + 1, NQ))
    per_core = []
    for c in range(NCORES):
        e0, e1 = starts[c], starts[c + 1]
        ne = e1 - e0
        assert ne <= mcp, f"core {c}: {ne} edges > capacity {mcp}"
        qg = np.full(mcp, (c + 1) * NQ - 1, np.int64)
        kg = np.zeros(mcp, np.int64)
        qg[:ne] = qi_s[e0:e1]
        kg[:ne] = ki_s[e0:e1]
        per_core.append((qg, kg, ne))
    return per_core


def _pick_cfg(inputs):
    qi = np.asarray(inputs["qk_pair_idxs"][0], np.int64)
    counts = np.bincount(qi // NQ, minlength=NCORES)
    need = int(np.ceil((counts.max() + 1) / 128.0))
    nb = max(((need + CHUNK - 1) // CHUNK) * CHUNK, 2 * CHUNK)
    wmax, kpmax = 1, 1
    for qg, kg, ne in _core_edges(inputs, nb):
        qlv = qg - (qg[-1] // NQ) * NQ
        b = qlv.reshape(nb, 128)
        delta = b - b[:, 0][:, None]
        wmax = max(wmax, int(delta.max()) + 1)
        qstarts = np.searchsorted(qlv[:ne], np.arange(NQ + 1))
        q0, q1 = qstarts[:-1], qstarts[1:]
        nz = q1 > q0
        if nz.any():
            npieces = ((q1[nz] - 1) // 128 - q0[nz] // 128 + 1).max()
            kpmax = max(kpmax, int(npieces))
    w = min(((wmax + 7) // 8) * 8, 128)
    kp = max(int(kpmax), 2)
    return nb, w, kp


def _prep(inputs, nb, W, KP):
    """Host-side sharding/layout prep. Pure indexing + dtype casts."""
    mcp = nb * 128
    x_F = np.asarray(inputs["x_F"], np.float32)
    b_qkv = np.asarray(inputs.get("b_qkv", np.zeros(3 * CH * 3 // 3)), np.float32)
    x_C = np.asarray(inputs["x_C"], np.float32)
    W_qkv = np.asarray(inputs["W_qkv"], np.float32)
    q_tables = np.asarray(inputs["q_tables"], np.float32)
    k_tables = np.asarray(inputs["k_tables"], np.float32)
    v_tables = np.asarray(inputs["v_tables"], np.float32)

    wkv_b = np.ascontiguousarray(W_qkv[:, CH:]).astype(BF16NP)
    wq_b = np.ascontiguousarray(W_qkv[:, :CH]).astype(BF16NP)
    lo = BINS // 2 - BQ // 2
    tabq = np.ascontiguousarray(
        q_tables[:, lo:lo + BQ].reshape(3 * BQ, CH)).astype(BF16NP)
    tabk = np.ascontiguousarray(
        k_tables[:, lo:lo + BQ].reshape(3 * BQ, CH)).astype(BF16NP)
    tabv = np.ascontiguousarray(
        v_tables[:, lo:lo + BQ].reshape(3 * BQ, CH)).astype(BF16NP)
    xchl = np.zeros((N, 8), BF16NP)
    xchl[:, 0:3] = x_C.astype(BF16NP)
    xchl[:, 3:6] = (x_C - xchl[:, 0:3].astype(np.float32)).astype(BF16NP)
    iota = np.tile(np.arange(lo, lo + BQ, dtype=np.float32), (128, 1))
    ident = np.eye(128, dtype=BF16NP)

    in_maps = []
    for c, (qg, kg, ne) in enumerate(_core_edges(inputs, nb)):
        kc, kr = kg // NQ, kg % NQ
        kg = np.where(kr < NQ // 2, kc * (NQ // 2) + kr,
                      N // 2 + kc * (NQ // 2) + (kr - NQ // 2))
        ql = qg - c * NQ
        ebias = np.full(mcp, PAD_BIAS, np.float32)
        ebias[:ne] = 0.0

        blocks_ql = ql.reshape(nb, 128)
        firsts = blocks_ql[:, 0]
        delta = blocks_ql - firsts[:, None]
        assert delta.min() >= 0 and delta.max() < W, f"core {c}: slot overflow {delta.max()}"
        smat = np.zeros((nb, 128, W), BF16NP)
        smat[np.arange(nb)[:, None], np.arange(128)[None, :], delta] = 1.0

        # pieces: for local query q, rows b*W + (q - firsts[b]) over the blocks
        # where q's edges live
        zrow = nb * W
        piece = np.full((NQ, KP), zrow, np.int64)
        qstarts = np.searchsorted(ql[:ne], np.arange(NQ + 1))
        for q in range(NQ):
            a0, a1 = qstarts[q], qstarts[q + 1]
            if a0 == a1:
                continue
            b0, b1 = a0 // 128, (a1 - 1) // 128
            nb_q = b1 - b0 + 1
            assert nb_q <= KP, f"core {c} query {q}: {nb_q} pieces"
            bs = np.arange(b0, b1 + 1)
            piece[q, :nb_q] = bs * W + (q - firsts[bs])
        # gather order: piece cc of query p (within a 128-query tile) at
        # flat position tile*KP*128 + cc*128 + p
        piece_fl = np.empty(NQ * KP, np.int64)
        for t in range(NQ // 128):
            blockv = piece[t * 128:(t + 1) * 128]  # [128, KP]
            piece_fl[t * KP * 128:(t + 1) * KP * 128] = blockv.T.reshape(-1)

        in_maps.append({
            "xTq": np.ascontiguousarray(x_F[c * NQ:(c + 1) * NQ].T).astype(BF16NP),
            "bqkv": b_qkv.reshape(1, 3 * CH).astype(BF16NP),
            "wkv": wkv_b, "wq": wq_b,
            "tabq": tabq, "tabk": tabk, "tabv": tabv,
            "xchlq": xchl[c * NQ:(c + 1) * NQ],
            "iota": iota, "ident": ident,
            "ki": _wrap_idx(kg), "qil": _wrap_idx(ql),
            "piece": _wrap_idx(piece_fl),
            "smat": smat,
            "ebias": np.ascontiguousarray(ebias.reshape(nb, 128).T),
        })
    return in_maps


LAST_RESULT = None


def benchmark(inputs, iters=20):
    """Time the 8-core kernel on HW: jit once (no donation), device_put
    inputs once, launch `iters` async executions, block at the end."""
    import time
    import jax
    from jax.sharding import Mesh, PartitionSpec
    from jax.experimental.shard_map import shard_map
    from concourse import bass2jax

    cfg = _pick_cfg(inputs)
    if cfg not in _CACHE:
        _CACHE[cfg] = _build(*cfg)
    nc = _CACHE[cfg]
    in_maps = _prep(inputs, *cfg)
    bass2jax.install_neuronx_cc_hook()

    partition_name = nc.partition_id_tensor.name if nc.partition_id_tensor else None
    in_names, out_names, out_avals, zero_outs = [], [], [], []
    for alloc in nc.m.functions[0].allocations:
        if not isinstance(alloc, mybir.MemoryLocationSet):
            continue
        name = alloc.memorylocations[0].name
        if alloc.kind == "ExternalInput":
            if name != partition_name:
                in_names.append(name)
        elif alloc.kind == "ExternalOutput":
            out_names.append(name)
            shape = tuple(alloc.tensor_shape)
            dtype = mybir.dt.np(alloc.dtype)
            out_avals.append(jax.core.ShapedArray(shape, dtype))
            zero_outs.append(np.zeros(shape, dtype))
    n_params = len(in_names)
    all_in = in_names + out_names
    if partition_name is not None:
        all_in.append(partition_name)

    def _body(*args):
        operands = list(args)
        if partition_name is not None:
            operands.append(bass2jax.partition_id_tensor())
        outs = bass2jax._bass_exec_p.bind(
            *operands,
            out_avals=tuple(out_avals),
            in_names=tuple(all_in),
            out_names=tuple(out_names),
            lowering_input_output_aliases=(),
            sim_require_finite=True,
            sim_require_nnan=True,
            nc=nc,
        )
        return tuple(outs)

    devices = jax.devices()[:NCORES]
    mesh = Mesh(np.asarray(devices), ("core",))
    nin = n_params + len(out_names)
    fn = jax.jit(
        shard_map(_body, mesh=mesh,
                  in_specs=(PartitionSpec("core"),) * nin,
                  out_specs=(PartitionSpec("core"),) * len(out_names),
                  check_rep=False),
        keep_unused=True,
    )
    concat_in = [
        np.concatenate([np.asarray(in_maps[c][nm]) for c in range(NCORES)], axis=0)
        for nm in in_names
    ]
    concat_zeros = [
        np.zeros((NCORES * z.shape[0], *z.shape[1:]), z.dtype) for z in zero_outs
    ]
    args = [jax.device_put(a) for a in concat_in + concat_zeros]
    out = fn(*args)
    jax.block_until_ready(out)
    t0 = time.perf_counter()
    for _ in range(iters):
        out = fn(*args)
    jax.block_until_ready(out)
    t1 = time.perf_counter()
    per_iter_ns = (t1 - t0) / iters * 1e9
    res = np.asarray(out[0]).reshape(NCORES, NQ, CH).reshape(N, CH)
    return per_iter_ns, res


def kernel(**inputs):
    global LAST_RESULT
    cfg = _pick_cfg(inputs)
    if cfg not in _CACHE:
        _CACHE[cfg] = _build(*cfg)
    nc = _CACHE[cfg]
    in_maps = _prep(inputs, *cfg)
    trace = bool(int(os.environ.get("CRPE_TRACE", "0")))
    res = run_bass_kernel_spmd(nc, in_maps, core_ids=list(range(NCORES)),
                               trace=trace)
    LAST_RESULT = res
    out = np.concatenate([res.results[c]["out"] for c in range(NCORES)], axis=0)
    return out.astype(np.float32)


# revision 71
# speedup vs baseline: 1.0460x; 1.0460x over previous
"""CRPE sparse attention kernel for 8 Trainium2 NeuronCores.

Strategy (graph/edge parallelism over query-sorted edges):
  * Host sorts the M edges by query index; core c owns all edges whose query
    lies in [1024c, 1024c+1024). Queries are disjoint per core, so the
    segment softmax and output rows are core-local; the host concatenates
    the 8 output slices.
  * P1: each core projects q|k|v (+bias) for its own 1024 rows; the k|v rows
    (augmented with bf16 hi/lo-split x_C coords for exact bin computation)
    are AllGathered so every core holds the full [8192, 1152] k|v table.
  * P2: edges in 128-wide blocks (edge -> SBUF partition), two blocks per
    iteration to halve DVE instruction count:
      - k|v and q rows fetched with dma_gather (2.25KB / 1.25KB bf16 rows),
      - relative-position bins in f32 (exact +2^23 floor trick); one fused
        is_equal builds all per-axis one-hots of a chunk (bins 4..43 only:
        x_C uniform in [0,1) keeps rel/0.05+24 inside [4,44)),
      - PE: transpose one-hots, matmul against the (120x512) tables for
        per-edge T_q/T_k/T_v rows; k rows are folded into T_q in PSUM via an
        identity-matmul accumulate,
      - DVE: logits = q.(k+T_q) + k.T_k per head with an interleaved
        product layout and a single fused reduce,
      - ACT: e = exp(logits + pad_bias); max-subtraction is skipped since
        logits are O(1) by construction and softmax is shift-invariant,
      - block-local segment reduction via one-hot scatter matmuls
        S^T @ [e*(v+T_v) | e] into W query slots; partials go to DRAM.
  * P3: per query, gather its <=KP partial rows, sum, divide by the summed
    denominator (+1e-20 so empty segments yield 0), write the output.
"""

import os

os.environ.setdefault("MYCRO_LOCAL_CACHE", "1")

import numpy as np
import ml_dtypes

import concourse.mybir as mybir
import concourse.bass as bass
import concourse.bacc as bacc
import concourse.tile as tile
from concourse.bass_utils import run_bass_kernel_spmd

F32 = mybir.dt.float32
BF16 = mybir.dt.bfloat16
I16 = mybir.dt.int16
ALU = mybir.AluOpType
ACTF = mybir.ActivationFunctionType
BF16NP = ml_dtypes.bfloat16

N = 8192
M_EDGES = 131072
C_IN = 512
H = 8
D = 64
CH = H * D          # 512
KVW = 2 * CH        # 1024
KVR = 1152          # kv row: k(512) | v(512) | xc_hi(3) | xc_lo(3) | pad -> 2304B
QR = 640            # q row: q(512) | xc_hi(3) | xc_lo(3) | pad -> 1280B
BINS = 48
BIN_SIZE = 0.05
BQ = 40             # effective bins 4..43 (x_C uniform [0,1) keeps rel in (4,44))
NCORES = 8
NQ = N // NCORES    # queries per core
CHUNK = 8           # 128-edge blocks per gather chunk
PAD_BIAS = -30000.0

_CACHE = {}


def _bcast(ap, n):
    """Append a stride-0 dim of size n to an AP (free-dim broadcast)."""
    return bass.AP(ap.tensor, ap.offset, list(ap.ap) + [[0, n]])


def _wrap_idx(a):
    """dma_gather index layout: wrapped in 16 partitions, replicated x8."""
    a = np.asarray(a, np.int16)
    assert len(a) % 16 == 0
    return np.ascontiguousarray(np.tile(a.reshape(-1, 16).T, (8, 1)))


def _build(nb, W, KP):
    """Build + compile the single-core SPMD program for NB=nb edge blocks."""
    mcp = nb * 128
    nchunks = nb // CHUNK
    zr = nb * W  # index of the all-zero partials row

    nc = bacc.Bacc("TRN2", target_bir_lowering=False, debug=False,
                   num_swdge_queues=4, num_devices=NCORES)

    # ---- I/O ----
    xTq = nc.dram_tensor("xTq", [C_IN, NQ], BF16, kind="ExternalInput")
    bqkv = nc.dram_tensor("bqkv", [1, 3 * CH], BF16, kind="ExternalInput")
    wkv = nc.dram_tensor("wkv", [C_IN, KVW], BF16, kind="ExternalInput")
    wq = nc.dram_tensor("wq", [C_IN, CH], BF16, kind="ExternalInput")
    tabq = nc.dram_tensor("tabq", [3 * BQ, CH], BF16, kind="ExternalInput")
    tabk = nc.dram_tensor("tabk", [3 * BQ, CH], BF16, kind="ExternalInput")
    tabv = nc.dram_tensor("tabv", [3 * BQ, CH], BF16, kind="ExternalInput")
    xchlq = nc.dram_tensor("xchlq", [NQ, 8], BF16, kind="ExternalInput")
    iota = nc.dram_tensor("iota", [128, BQ], F32, kind="ExternalInput")
    ident = nc.dram_tensor("ident", [128, 128], BF16, kind="ExternalInput")
    ki_d = nc.dram_tensor("ki", [128, mcp // 16], I16, kind="ExternalInput")
    qil_d = nc.dram_tensor("qil", [128, mcp // 16], I16, kind="ExternalInput")
    piece_d = nc.dram_tensor("piece", [128, NQ * KP // 16], I16, kind="ExternalInput")
    smat = nc.dram_tensor("smat", [nb, 128, W], BF16, kind="ExternalInput")
    ebias_d = nc.dram_tensor("ebias", [128, nb], F32, kind="ExternalInput")
    out_d = nc.dram_tensor("out", [NQ, CH], F32, kind="ExternalOutput")

    # ---- internal DRAM ----
    kv_mine = nc.dram_tensor("kv_m", [NQ, KVR], BF16)
    kv_dram = nc.dram_tensor("kv_i", [N, KVR], BF16, addr_space="Shared")
    q_dram = nc.dram_tensor("q_i", [NQ, QR], BF16)
    part_dram = nc.dram_tensor("part_i", [nb * W + 1, 576], F32)

    with tile.TileContext(nc) as tc:
        with tc.tile_pool(name="const", bufs=1) as cp:
            wkv_sb = cp.tile([128, 4, KVW], BF16)
            nc.sync.dma_start(wkv_sb[:], wkv[:].rearrange("(c p) n -> p c n", p=128))
            wq_sb = cp.tile([128, 4, CH], BF16)
            nc.sync.dma_start(wq_sb[:], wq[:].rearrange("(c p) n -> p c n", p=128))
            tq_sb = cp.tile([3 * BQ, CH], BF16)
            nc.sync.dma_start(tq_sb[:], tabq[:])
            tk_sb = cp.tile([3 * BQ, CH], BF16)
            nc.sync.dma_start(tk_sb[:], tabk[:])
            tv_sb = cp.tile([3 * BQ, CH], BF16)
            nc.sync.dma_start(tv_sb[:], tabv[:])
            iota_sb = cp.tile([128, BQ], F32)
            nc.sync.dma_start(iota_sb[:], iota[:])
            ident_sb = cp.tile([128, 128], BF16)
            nc.sync.dma_start(ident_sb[:], ident[:])
            ki_sb = cp.tile([128, mcp // 16], I16)
            nc.sync.dma_start(ki_sb[:], ki_d[:])
            qil_sb = cp.tile([128, mcp // 16], I16)
            nc.sync.dma_start(qil_sb[:], qil_d[:])
            piece_sb = cp.tile([128, NQ * KP // 16], I16)
            nc.sync.dma_start(piece_sb[:], piece_d[:])
            ebias_sb = cp.tile([128, nb], F32)
            nc.sync.dma_start(ebias_sb[:], ebias_d[:])
            b_sb = cp.tile([1, 3 * CH], BF16)
            nc.sync.dma_start(b_sb[:], bqkv[:])
            ones1 = cp.tile([1, 128], BF16)
            nc.vector.memset(ones1[:], 1.0)

            # ---------------- P1: projections ----------------
            with (
                tc.tile_pool(name="p1sb", bufs=3) as p1,
                tc.tile_pool(name="p1ps", bufs=2, space="PSUM") as p1p,
            ):
                # one fused loop: q|k|v from a single xT load per row tile;
                # q_dram completes before the collective so P2's q gathers
                # can prefetch under it
                for r in range(NQ // 128):
                    xt_t = p1.tile([128, 4, 128], BF16, tag="xt")
                    nc.sync.dma_start(
                        xt_t[:],
                        xTq[:, r * 128:(r + 1) * 128].rearrange(
                            "(c p) m -> p c m", p=128
                        ),
                    )
                    psq = p1p.tile([128, CH], F32, tag="psq")
                    for c in range(4):
                        nc.tensor.matmul(
                            psq[:], xt_t[:, c, :], wq_sb[:, c, :],
                            start=(c == 0), stop=False,
                        )
                    nc.tensor.matmul(psq[:], ones1[:], b_sb[:, 0:CH],
                                     start=False, stop=True)
                    q_sb = p1.tile([128, QR], BF16, tag="qo")
                    # fold the D^-0.5 query scaling into the PSUM->SBUF copy
                    nc.scalar.activation(q_sb[:, 0:CH], psq[:], ACTF.Copy,
                                         scale=float(D) ** -0.5)
                    nc.sync.dma_start(q_sb[:, CH:CH + 8],
                                      xchlq[r * 128:(r + 1) * 128, :])
                    nc.gpsimd.memset(q_sb[:, CH + 8:QR], 0.0)
                    nc.sync.dma_start(q_dram[r * 128:(r + 1) * 128, :], q_sb[:])

                    ps = p1p.tile([128, KVW], F32, tag="pskv")
                    for c in range(4):
                        nc.tensor.matmul(
                            ps[:, 0:CH], xt_t[:, c, :], wkv_sb[:, c, 0:CH],
                            start=(c == 0), stop=False,
                        )
                        nc.tensor.matmul(
                            ps[:, CH:KVW], xt_t[:, c, :], wkv_sb[:, c, CH:KVW],
                            start=(c == 0), stop=False,
                        )
                    # + b_kv broadcast to all rows (K=1 outer product)
                    nc.tensor.matmul(ps[:, 0:CH], ones1[:], b_sb[:, CH:KVW],
                                     start=False, stop=True)
                    nc.tensor.matmul(ps[:, CH:KVW], ones1[:], b_sb[:, KVW:KVW + CH],
                                     start=False, stop=True)
                    kv_sb = p1.tile([128, KVR], BF16, tag="kvo")
                    nc.scalar.copy(kv_sb[:, 0:KVW], ps[:])
                    nc.sync.dma_start(kv_sb[:, KVW:KVW + 8],
                                      xchlq[r * 128:(r + 1) * 128, :])
                    nc.gpsimd.memset(kv_sb[:, KVW + 8:KVR], 0.0)
                    nc.sync.dma_start(kv_mine[r * 128:(r + 1) * 128, :], kv_sb[:])
                # two half AllGathers: the first overlaps the second half of
                # the projection loop. Row layout becomes half-blocked
                # (ki indices are remapped accordingly on the host).
                nc.gpsimd.collective_compute(
                    "AllGather", ALU.bypass,
                    replica_groups=[list(range(NCORES))],
                    ins=[kv_mine[0:NQ // 2, :]], outs=[kv_dram[0:N // 2, :]],
                )
                nc.gpsimd.collective_compute(
                    "AllGather", ALU.bypass,
                    replica_groups=[list(range(NCORES))],
                    ins=[kv_mine[NQ // 2:NQ, :]], outs=[kv_dram[N // 2:N, :]],
                )

            # partials pad columns (520:576) are gathered in P3 but never
            # summed; zero-fill them once so the rows are fully defined
            with tc.tile_pool(name="zf", bufs=1) as zf:
                z56 = zf.tile([128, 56], F32)
                nc.vector.memset(z56[:], 0.0)
                nrows = nb * W + 1
                for r0 in range(0, nrows, 128):
                    n = min(128, nrows - r0)
                    nc.sync.dma_start(part_dram[r0:r0 + n, 520:576], z56[0:n, :])

            # ---------------- P2: edge blocks ----------------
            with (
                tc.tile_pool(name="p2g", bufs=3) as pg_pool,
                tc.tile_pool(name="p2w", bufs=4) as pw,
                tc.tile_pool(name="p2ps", bufs=2, space="PSUM") as pps,
                tc.tile_pool(name="p2ps1", bufs=2, space="PSUM") as pps1,
                tc.tile_pool(name="p2ps2", bufs=1, space="PSUM") as pps2,
            ):
                for g in range(nchunks):
                    i0 = g * CHUNK * 8  # idx column offset (16 idx per column)
                    hc = CHUNK // 2
                    kvg = pg_pool.tile([128, CHUNK, KVR], BF16, tag="kvg")
                    qg = pg_pool.tile([128, CHUNK, QR], BF16, tag="qg")
                    for hf in range(2):
                        nc.gpsimd.dma_gather(
                            kvg[:, hf * hc:(hf + 1) * hc, :], kv_dram[:],
                            ki_sb[:, i0 + hf * hc * 8:i0 + (hf + 1) * hc * 8],
                            hc * 128, hc * 128, KVR, queue_num=0,
                        )
                        nc.gpsimd.dma_gather(
                            qg[:, hf * hc:(hf + 1) * hc, :], q_dram[:],
                            qil_sb[:, i0 + hf * hc * 8:i0 + (hf + 1) * hc * 8],
                            hc * 128, hc * 128, QR, queue_num=0,
                        )
                    s_t = pg_pool.tile([128, CHUNK, W], BF16, tag="smat")
                    nc.sync.dma_start(
                        s_t[:],
                        smat[g * CHUNK:(g + 1) * CHUNK].rearrange("b p w -> p b w"),
                    )

                    # bin indices for the whole chunk, f32-exact floor of
                    # (xcq-xck)/BIN_SIZE + BINS/2 with xc = hi + lo (bf16 pair)
                    d6 = pw.tile([128, CHUNK, 6], F32, tag="d6")
                    nc.vector.tensor_tensor(
                        d6[:], qg[:, :, CH:CH + 6], kvg[:, :, KVW:KVW + 6],
                        op=ALU.subtract,
                    )
                    rel = pw.tile([128, CHUNK, 3], F32, tag="rel")
                    nc.vector.tensor_tensor(
                        rel[:], d6[:, :, 0:3], d6[:, :, 3:6], op=ALU.add)
                    nc.vector.tensor_scalar(
                        rel[:], rel[:], 1.0 / BIN_SIZE, BINS / 2.0,
                        op0=ALU.mult, op1=ALU.add,
                    )
                    flo = pw.tile([128, CHUNK, 3], F32, tag="flo")
                    # exact floor: y = (x + 2^23) - 2^23, then y -= (y > x)
                    nc.vector.tensor_scalar(flo[:], rel[:], float(2 ** 23),
                                            float(2 ** 23), op0=ALU.add,
                                            op1=ALU.subtract)
                    gt = d6[:, :, 0:3]  # d6 is dead after rel; reuse as scratch
                    nc.vector.tensor_tensor(gt, flo[:], rel[:], op=ALU.is_gt)
                    nc.vector.tensor_tensor(flo[:], flo[:], gt, op=ALU.subtract)
                    nc.vector.tensor_scalar(
                        flo[:], flo[:], float(BINS / 2 - BQ / 2), float(BINS / 2 + BQ / 2 - 1),
                        op0=ALU.max, op1=ALU.min,
                    )
                    # all CHUNK x 3 one-hots in a single compare:
                    # ohc[p, b, a*BQ+j] = (flo[p, b, a] == iota[j]);
                    # cols 120:128 are zero padding for the DMA transpose
                    ohc = pw.tile([128, CHUNK, 128], BF16, tag="ohc")
                    flo_b = bass.AP(flo[:].tensor, flo[:].offset,
                                    list(flo[:].ap) + [[0, BQ]])
                    iota_v = bass.AP(iota_sb[:].tensor, iota_sb[:].offset,
                                     [iota_sb[:].ap[0], [0, CHUNK], [0, 3],
                                      [1, BQ]])
                    nc.vector.tensor_tensor(
                        ohc[:, :, 0:3 * BQ].rearrange("p b (a j) -> p b a j", a=3),
                        flo_b, iota_v, op=ALU.is_equal,
                    )
                    nc.gpsimd.memset(ohc[:, :, 3 * BQ:128], 0.0)

                    for b in range(0, CHUNK, 2):
                        blk = g * CHUNK + b
                        oht_ps = pps1.tile([128, 2, 128], BF16, tag="ohtps")
                        for j in range(2):
                            nc.tensor.transpose(
                                oht_ps[0:3 * BQ, j, :], ohc[:, b + j, 0:3 * BQ],
                                ident_sb[:],
                            )
                        oht_f = pw.tile([128, 2, 128], BF16, tag="oht")
                        nc.scalar.copy(oht_f[0:3 * BQ, :, :], oht_ps[0:3 * BQ, :, :])
                        oht = [oht_f[0:3 * BQ, 0, :], oht_f[0:3 * BQ, 1, :]]

                        kv2 = kvg[:, b:b + 2, :]
                        # serially reused 2-bank PSUM pair tile; k rows are
                        # added into T_q on PE via identity-matmul accumulation
                        t_q = pps.tile([128, 2, CH], F32, tag="T")
                        for j in range(2):
                            nc.tensor.matmul(t_q[:, j, :], oht[j], tq_sb[:],
                                             start=True, stop=False)
                            nc.tensor.matmul(t_q[:, j, :], ident_sb[:],
                                             kvg[:, b + j, 0:CH],
                                             start=False, stop=True)
                        t_k = pps.tile([128, 2, CH], F32, tag="T")
                        nc.tensor.matmul(t_k[:, 0, :], oht[0], tk_sb[:])
                        nc.tensor.matmul(t_k[:, 1, :], oht[1], tk_sb[:])
                        tkb = pw.tile([128, 2, CH], BF16, tag="tkb")
                        nc.scalar.copy(tkb[:], t_k[:])
                        tqb = pw.tile([128, 2, CH], BF16, tag="tqb")
                        nc.scalar.copy(tqb[:], t_q[:])
                        # pa/pb interleaved per head -> one fused reduce
                        pab = pw.tile([128, 2, 2 * CH], BF16, tag="pab")
                        pab_v = pab[:].rearrange(
                            "p b (h s d) -> p b h s d", s=2, d=D)
                        nc.vector.tensor_tensor(
                            pab_v[:, :, :, 0, :],
                            qg[:, b:b + 2, 0:CH].rearrange(
                                "p b (h d) -> p b h d", d=D),
                            tqb[:].rearrange("p b (h d) -> p b h d", d=D),
                            op=ALU.mult,
                        )
                        nc.vector.tensor_tensor(
                            pab_v[:, :, :, 1, :],
                            kv2[:, :, 0:CH].rearrange("p b (h d) -> p b h d", d=D),
                            tkb[:].rearrange("p b (h d) -> p b h d", d=D),
                            op=ALU.mult,
                        )
                        lg = pw.tile([128, 2, H], F32, tag="lg")
                        nc.vector.tensor_reduce(
                            lg[:], pab[:].rearrange("p b (h x) -> p b h x", x=2 * D),
                            axis=mybir.AxisListType.X, op=ALU.add,
                        )

                        srhs = pw.tile([128, 2, CH + 8], BF16, tag="srhs")
                        for j in range(2):
                            nc.scalar.activation(
                                srhs[:, j, CH:CH + H], lg[:, j, :], ACTF.Exp,
                                bias=ebias_sb[:, blk + j:blk + j + 1],
                            )
                        t_v = pps.tile([128, 2, CH], F32, tag="T")
                        nc.tensor.matmul(t_v[:, 0, :], oht[0], tv_sb[:])
                        nc.tensor.matmul(t_v[:, 1, :], oht[1], tv_sb[:])
                        tvb = pw.tile([128, 2, CH], BF16, tag="tvb")
                        nc.scalar.copy(tvb[:], t_v[:])
                        vs = pw.tile([128, 2, CH], BF16, tag="vs")
                        nc.vector.tensor_tensor(
                            vs[:], kv2[:, :, CH:KVW], tvb[:], op=ALU.add
                        )
                        nc.vector.tensor_tensor(
                            srhs[:, :, 0:CH].rearrange(
                                "p b (h d) -> p b h d", d=D),
                            vs[:].rearrange("p b (h d) -> p b h d", d=D),
                            _bcast(srhs[:, :, CH:CH + H], D),
                            op=ALU.mult,
                        )

                        for j in range(2):
                            sc = pps2.tile([W, 520], F32, tag="sc")
                            nc.tensor.matmul(sc[:, 0:CH], s_t[:, b + j, :],
                                             srhs[:, j, 0:CH])
                            nc.tensor.matmul(sc[:, CH:CH + H], s_t[:, b + j, :],
                                             srhs[:, j, CH:CH + H])
                            part = pw.tile([W, 520], F32, tag="part")
                            nc.scalar.copy(part[:], sc[:, 0:520])
                            nc.sync.dma_start(
                                part_dram[(blk + j) * W:(blk + j + 1) * W, 0:520],
                                part[:],
                            )

            # ---------------- P3: merge partials, divide ----------------
            with tc.tile_pool(name="p3", bufs=2) as p3:
                z = p3.tile([1, 576], F32, tag="z")
                nc.vector.memset(z[:], 0.0)
                nc.sync.dma_start(part_dram[zr:zr + 1, :], z[:])
                for t in range(NQ // 128):
                    pcs = p3.tile([128, KP, 576], F32, tag="pcs")
                    nc.gpsimd.dma_gather(
                        pcs[:], part_dram[:],
                        piece_sb[:, t * KP * 8:(t + 1) * KP * 8],
                        KP * 128, KP * 128, 576, queue_num=0,
                    )
                    s1 = p3.tile([128, 520], F32, tag="s1")
                    nc.vector.tensor_tensor(
                        s1[:], pcs[:, 0, 0:520], pcs[:, 1, 0:520], op=ALU.add
                    )
                    for j in range(2, KP):
                        nc.vector.tensor_tensor(
                            s1[:], s1[:], pcs[:, j, 0:520], op=ALU.add
                        )
                    rc = p3.tile([128, H], F32, tag="rc")
                    nc.vector.tensor_scalar_add(rc[:], s1[:, CH:CH + H], 1e-20)
                    nc.vector.reciprocal(rc[:], rc[:])
                    o_t = p3.tile([128, CH], F32, tag="o")
                    nc.vector.tensor_tensor(
                        o_t[:].rearrange("p (h d) -> p h d", d=D),
                        s1[:, 0:CH].rearrange("p (h d) -> p h d", d=D),
                        _bcast(rc[:], D),
                        op=ALU.mult,
                    )
                    nc.sync.dma_start(out_d[t * 128:(t + 1) * 128, :], o_t[:])

    nc.compile()
    return nc


def _core_edges(inputs, nb):
    """Sorted, padded per-core (qi_local, ki_global) index arrays."""
    mcp = nb * 128
    pairs = np.asarray(inputs["qk_pair_idxs"])
    qi = pairs[0].astype(np.int64)
    ki = pairs[1].astype(np.int64)
    order = np.argsort(qi, kind="stable")
    qi_s = qi[order]
    ki_s = ki[order]
    starts = np.searchsorted(qi_s, np.arange(0, N + 1, NQ))
    per_core = []
    for c in range(NCORES):
        e0, e1 = starts[c], starts[c + 1]
        ne = e1 - e0
        assert ne <= mcp, f"core {c}: {ne} edges > capacity {mcp}"
        qg = np.full(mcp, (c + 1) * NQ - 1, np.int64)
        kg = np.zeros(mcp, np.int64)
        qg[:ne] = qi_s[e0:e1]
        kg[:ne] = ki_s[e0:e1]
        per_core.append((qg, kg, ne))
    return per_core


def _pick_cfg(inputs):
    qi = np.asarray(inputs["qk_pair_idxs"][0], np.int64)
    counts = np.bincount(qi // NQ, minlength=NCORES)
    need = int(np.ceil((counts.max() + 1) / 128.0))
    nb = max(((need + CHUNK - 1) // CHUNK) * CHUNK, 2 * CHUNK)
    wmax, kpmax = 1, 1
    for qg, kg, ne in _core_edges(inputs, nb):
        qlv = qg - (qg[-1] // NQ) * NQ
        b = qlv.reshape(nb, 128)
        delta = b - b[:, 0][:, None]
        wmax = max(wmax, int(delta.max()) + 1)
        qstarts = np.searchsorted(qlv[:ne], np.arange(NQ + 1))
        q0, q1 = qstarts[:-1], qstarts[1:]
        nz = q1 > q0
        if nz.any():
            npieces = ((q1[nz] - 1) // 128 - q0[nz] // 128 + 1).max()
            kpmax = max(kpmax, int(npieces))
    w = min(((wmax + 7) // 8) * 8, 128)
    kp = max(int(kpmax), 2)
    return nb, w, kp


def _prep(inputs, nb, W, KP):
    """Host-side sharding/layout prep. Pure indexing + dtype casts."""
    mcp = nb * 128
    x_F = np.asarray(inputs["x_F"], np.float32)
    b_qkv = np.asarray(inputs.get("b_qkv", np.zeros(3 * CH * 3 // 3)), np.float32)
    x_C = np.asarray(inputs["x_C"], np.float32)
    W_qkv = np.asarray(inputs["W_qkv"], np.float32)
    q_tables = np.asarray(inputs["q_tables"], np.float32)
    k_tables = np.asarray(inputs["k_tables"], np.float32)
    v_tables = np.asarray(inputs["v_tables"], np.float32)

    wkv_b = np.ascontiguousarray(W_qkv[:, CH:]).astype(BF16NP)
    wq_b = np.ascontiguousarray(W_qkv[:, :CH]).astype(BF16NP)
    lo = BINS // 2 - BQ // 2
    tabq = np.ascontiguousarray(
        q_tables[:, lo:lo + BQ].reshape(3 * BQ, CH)).astype(BF16NP)
    tabk = np.ascontiguousarray(
        k_tables[:, lo:lo + BQ].reshape(3 * BQ, CH)).astype(BF16NP)
    tabv = np.ascontiguousarray(
        v_tables[:, lo:lo + BQ].reshape(3 * BQ, CH)).astype(BF16NP)
    xchl = np.zeros((N, 8), BF16NP)
    xchl[:, 0:3] = x_C.astype(BF16NP)
    xchl[:, 3:6] = (x_C - xchl[:, 0:3].astype(np.float32)).astype(BF16NP)
    iota = np.tile(np.arange(lo, lo + BQ, dtype=np.float32), (128, 1))
    ident = np.eye(128, dtype=BF16NP)

    in_maps = []
    for c, (qg, kg, ne) in enumerate(_core_edges(inputs, nb)):
        kc, kr = kg // NQ, kg % NQ
        kg = np.where(kr < NQ // 2, kc * (NQ // 2) + kr,
                      N // 2 + kc * (NQ // 2) + (kr - NQ // 2))
        ql = qg - c * NQ
        ebias = np.full(mcp, PAD_BIAS, np.float32)
        ebias[:ne] = 0.0

        blocks_ql = ql.reshape(nb, 128)
        firsts = blocks_ql[:, 0]
        delta = blocks_ql - firsts[:, None]
        assert delta.min() >= 0 and delta.max() < W, f"core {c}: slot overflow {delta.max()}"
        smat = np.zeros((nb, 128, W), BF16NP)
        smat[np.arange(nb)[:, None], np.arange(128)[None, :], delta] = 1.0

        # pieces: for local query q, rows b*W + (q - firsts[b]) over the blocks
        # where q's edges live
        zrow = nb * W
        piece = np.full((NQ, KP), zrow, np.int64)
        qstarts = np.searchsorted(ql[:ne], np.arange(NQ + 1))
        for q in range(NQ):
            a0, a1 = qstarts[q], qstarts[q + 1]
            if a0 == a1:
                continue
            b0, b1 = a0 // 128, (a1 - 1) // 128
            nb_q = b1 - b0 + 1
            assert nb_q <= KP, f"core {c} query {q}: {nb_q} pieces"
            bs = np.arange(b0, b1 + 1)
            piece[q, :nb_q] = bs * W + (q - firsts[bs])
        # gather order: piece cc of query p (within a 128-query tile) at
        # flat position tile*KP*128 + cc*128 + p
        piece_fl = np.empty(NQ * KP, np.int64)
        for t in range(NQ // 128):
            blockv = piece[t * 128:(t + 1) * 128]  # [128, KP]
            piece_fl[t * KP * 128:(t + 1) * KP * 128] = blockv.T.reshape(-1)

        in_maps.append({
            "xTq": np.ascontiguousarray(x_F[c * NQ:(c + 1) * NQ].T).astype(BF16NP),
            "bqkv": b_qkv.reshape(1, 3 * CH).astype(BF16NP),
            "wkv": wkv_b, "wq": wq_b,
            "tabq": tabq, "tabk": tabk, "tabv": tabv,
            "xchlq": xchl[c * NQ:(c + 1) * NQ],
            "iota": iota, "ident": ident,
            "ki": _wrap_idx(kg), "qil": _wrap_idx(ql),
            "piece": _wrap_idx(piece_fl),
            "smat": smat,
            "ebias": np.ascontiguousarray(ebias.reshape(nb, 128).T),
        })
    return in_maps


LAST_RESULT = None


def benchmark(inputs, iters=20):
    """Time the 8-core kernel on HW: jit once (no donation), device_put
    inputs once, launch `iters` async executions, block at the end."""
    import time
    import jax
    from jax.sharding import Mesh, PartitionSpec
    from jax.experimental.shard_map import shard_map
    from concourse import bass2jax

    cfg = _pick_cfg(inputs)
    if cfg not in _CACHE:
        _CACHE[cfg] = _build(*cfg)
    nc = _CACHE[cfg]
    in_maps = _prep(inputs, *cfg)
    bass2jax.install_neuronx_cc_hook()

    partition_name = nc.partition_id_tensor.name if nc.partition_id_tensor else None
    in_names, out_names, out_avals, zero_outs = [], [], [], []
    for alloc in nc.m.functions[0].allocations:
        if not isinstance(alloc, mybir.MemoryLocationSet):
            continue
        name = alloc.memorylocations[0].name
        if alloc.kind == "ExternalInput":
            if name != partition_name:
                in_names.append(name)
        elif alloc.kind == "ExternalOutput":
            out_names.append(name)
            shape = tuple(alloc.tensor_shape)
            dtype = mybir.dt.np(alloc.dtype)
            out_avals.append(jax.core.ShapedArray(shape, dtype))
            zero_outs.append(np.zeros(shape, dtype))
    n_params = len(in_names)
    all_in = in_names + out_names
    if partition_name is not None:
        all_in.append(partition_name)

    def _body(*args):
        operands = list(args)
        if partition_name is not None:
            operands.append(bass2jax.partition_id_tensor())
        outs = bass2jax._bass_exec_p.bind(
            *operands,
            out_avals=tuple(out_avals),
            in_names=tuple(all_in),
            out_names=tuple(out_names),
            lowering_input_output_aliases=(),
            sim_require_finite=True,
            sim_require_nnan=True,
            nc=nc,
        )
        return tuple(outs)

    devices = jax.devices()[:NCORES]
    mesh = Mesh(np.asarray(devices), ("core",))
    nin = n_params + len(out_names)
    fn = jax.jit(
        shard_map(_body, mesh=mesh,
                  in_specs=(PartitionSpec("core"),) * nin,
                  out_specs=(PartitionSpec("core"),) * len(out_names),
                  check_rep=False),
        keep_unused=True,
    )
    concat_in = [
        np.concatenate([np.asarray(in_maps[c][nm]) for c in range(NCORES)], axis=0)
        for nm in in_names
    ]
    concat_zeros = [
        np.zeros((NCORES * z.shape[0], *z.shape[1:]), z.dtype) for z in zero_outs
    ]
    args = [jax.device_put(a) for a in concat_in + concat_zeros]
    out = fn(*args)
    jax.block_until_ready(out)
    t0 = time.perf_counter()
    for _ in range(iters):
        out = fn(*args)
    jax.block_until_ready(out)
    t1 = time.perf_counter()
    per_iter_ns = (t1 - t0) / iters * 1e9
    res = np.asarray(out[0]).reshape(NCORES, NQ, CH).reshape(N, CH)
    return per_iter_ns, res


def kernel(**inputs):
    global LAST_RESULT
    cfg = _pick_cfg(inputs)
    if cfg not in _CACHE:
        _CACHE[cfg] = _build(*cfg)
    nc = _CACHE[cfg]
    in_maps = _prep(inputs, *cfg)
    trace = bool(int(os.environ.get("CRPE_TRACE", "0")))
    res = run_bass_kernel_spmd(nc, in_maps, core_ids=list(range(NCORES)),
                               trace=trace)
    LAST_RESULT = res
    out = np.concatenate([res.results[c]["out"] for c in range(NCORES)], axis=0)
    return out.astype(np.float32)
